# revision 29
# baseline (speedup 1.0000x reference)
"""BitLinear (ternary-weight linear) kernel for Trainium2, 8 NeuronCores.

Computation:  out = x @ (w_ternary * scale)^T
  where scale = max(mean(|weight|), 1e-5)
        w_ternary = clip(round(weight / scale), -1.0, 1.0)  in {-1, 0, 1}

Strategy (data-parallel over batch, 1 batch row per core):
  - Host: quantize the weight to ternary (bit-exact mirror of the jnp
    reference), and split x*(64*scale) into an fp8-e4m3 hi/lo pair
    (x64 = hi + lo exactly to ~2^-9 relative).  The 64*scale folding keeps
    the fp8 values ~N(0,1), far from the e4m3 subnormal range; the exact
    power-of-two 2^-6 is unfolded on device in the output copy.
    Both planes are packed k-major per 128-row block so the device needs
    no transposes or casts at all:
      xp[b, kp, g, s] = plane_g[b*128+s, ks_g*128+kp]
    with g = 0..7 the hi plane (ks = g) and g = 8..15 the lo plane
    (ks = g-8).  Each block slab is a fully contiguous 256 KB DMA.
  - Device: pure fp8 DoubleRow matmuls (2 k-slices per instruction, 0.5
    cycles/row): per 128-row block, 8 stationary lhsT pair-tiles
    [128, 2, 128], each streamed against both 512-wide output halves of
    the un-duplicated fp8 weight, accumulating K_eff = 2048 (hi+lo) into
    two PSUM banks.  The scalar engine copies PSUM -> SBUF bf16 with the
    exact 1/64 scale, and the result DMAs out as bf16 (upcast to fp32 on
    host).  rel err vs the fp32 reference ~3.0e-3.
"""

import numpy as np
import ml_dtypes

B, S, IN, OUT = 8, 8192, 1024, 1024
N_CORES = 8
P = 128
S_BLOCKS = S // P    # 64
K_TILES = IN // P    # 8
LO_SLICES = [4, 5, 6, 7]   # k-slices that get the fp8 lo correction
G = K_TILES + len(LO_SLICES)  # 12 packed k-groups: hi plane + partial lo
EPS = 1e-5

F8 = ml_dtypes.float8_e4m3
BF16 = ml_dtypes.bfloat16

_compiled = None


def _build():
    import concourse.bacc as bacc
    import concourse.mybir as mybir
    import concourse.tile as tile

    F8D = mybir.dt.float8e4
    F32 = mybir.dt.float32
    BF = mybir.dt.bfloat16
    DR = mybir.MatmulPerfMode.DoubleRow

    nc = bacc.Bacc()
    xp = nc.declare_dram_parameter("xp", [S, G * P], F8D, isOutput=False)
    wt = nc.declare_dram_parameter("wt", [IN, OUT], F8D, isOutput=False)
    out = nc.declare_dram_parameter("out", [S, OUT], BF, isOutput=True)

    # DR pair schedule: (x-group pair start, weight k-slice pair start).
    # Groups 0..7 are the hi plane (ks = g); groups 8..11 are the lo plane
    # for k-slices LO_SLICES = 4..7.
    pairs = [(0, 0), (2, 2), (4, 4), (6, 6), (8, 4), (10, 6)]

    with tile.TileContext(nc) as tc:
        with (
            tc.tile_pool(name="const", bufs=1) as constp,
            tc.tile_pool(name="xn", bufs=3) as xnp,
            tc.tile_pool(name="outp", bufs=8) as outp,
            tc.tile_pool(name="pso", bufs=4, space="PSUM") as pso,
        ):
            def load_x(b):
                t = xnp.tile([P, G, P], F8D, tag="xn", name=f"xn_{b}")
                nc.sync.dma_start(out=t, in_=xp[b * P:(b + 1) * P, :])
                return t

            # Block 0's x goes first on the sync ring; the fp8 weight
            # streams concurrently on the gpsimd ring as 8 fully contiguous
            # 128 KB chunks (each dma_start costs ~650 ns of ring issue
            # time, so fewer, bigger chunks win).
            xt0 = load_x(0)
            wt_sb = constp.tile([P, K_TILES, OUT], F8D)
            for k in range(K_TILES):
                nc.gpsimd.dma_start(
                    out=wt_sb[:, k:k + 1, :],
                    in_=wt[k * P:(k + 1) * P, :],
                )

            for b in range(S_BLOCKS):
                xt = xt0 if b == 0 else load_x(b)

                # one 2-bank PSUM tile per block: halves h=0/1 accumulate in
                # adjacent banks and drain with a single 1024-wide Act copy
                po = pso.tile([P, 2, 512], F32, tag="pso", name=f"po{b}")
                # 6 stationary pair-tiles; each serves both output halves
                # back-to-back so weight loads are halved.
                for t, (xg, wk) in enumerate(pairs):
                    for h in range(2):
                        nc.tensor.matmul(
                            po[:, h, :],
                            lhsT=xt[:, xg:xg + 2, :],
                            rhs=wt_sb[:, wk:wk + 2, h * 512:(h + 1) * 512],
                            start=(t == 0),
                            stop=(t == len(pairs) - 1),
                            perf_mode=DR,
                        )

                ob = outp.tile([P, OUT], BF, tag="ob", name=f"ob_{b}")
                nc.scalar.activation(
                    ob, po,
                    mybir.ActivationFunctionType.Copy,
                    scale=1.0 / 64.0,
                )
                # outputs alternate between the gpsimd and scalar rings, so
                # the sync ring stays dedicated to the x prefetch and no
                # output backlog builds on one queue.  On the scalar ring
                # the DMA queues right behind this block's Act copies.  The
                # last block rides the by-then-idle sync ring: its software
                # queue finishes in ~0.5 us where gpsimd's hardware queue
                # has a ~4 us end-to-end transfer latency.
                if b == S_BLOCKS - 1:
                    ring = nc.sync
                else:
                    ring = nc.gpsimd if b % 2 == 0 else nc.scalar
                ring.dma_start(out=out[b * P:(b + 1) * P, :], in_=ob)
    nc.finalize()
    return nc


def _get_compiled():
    global _compiled
    if _compiled is None:
        _compiled = _build()
    return _compiled


def quantize_host(weight: np.ndarray):
    """Mirror of the reference ste_quantize, done on host in fp32.

    The mean is computed in float64 then rounded to fp32 so it tracks the
    true mean more closely than any fp32 summation order.
    """
    scale = np.float32(max(np.mean(np.abs(weight), dtype=np.float64), EPS))
    w_t = np.clip(np.round(weight / scale), -1.0, 1.0).astype(np.float32)
    return w_t, scale


def pack_weight(w_t: np.ndarray) -> np.ndarray:
    """Ternary weight [out, in] -> fp8 transposed [in, out]."""
    return np.ascontiguousarray(w_t.T).astype(F8)


def pack_x_core(xc: np.ndarray, c64: np.float32) -> np.ndarray:
    """One core's x [S, IN] fp32 -> packed fp8 hi + partial lo [S, G*P]."""
    xs = xc * c64
    hi = xs.astype(F8)
    lo_cols = np.concatenate(
        [np.arange(sl * P, (sl + 1) * P) for sl in LO_SLICES])
    lo = (xs[:, lo_cols] - hi.astype(np.float32)[:, lo_cols]).astype(F8)
    xp = np.empty((S_BLOCKS, P, G, P), dtype=F8)
    xp[:, :, 0:K_TILES, :] = hi.reshape(
        S_BLOCKS, P, K_TILES, P).transpose(0, 3, 2, 1)
    xp[:, :, K_TILES:G, :] = lo.reshape(
        S_BLOCKS, P, len(LO_SLICES), P).transpose(0, 3, 2, 1)
    return xp.reshape(S, G * P)


def make_in_maps(x: np.ndarray, weight: np.ndarray):
    x = np.asarray(x, dtype=np.float32)
    weight = np.asarray(weight, dtype=np.float32)
    assert x.shape == (B, S, IN) and weight.shape == (OUT, IN)
    w_t, scale = quantize_host(weight)
    wt8 = pack_weight(w_t)
    c64 = np.float32(64.0) * scale
    from concurrent.futures import ThreadPoolExecutor
    with ThreadPoolExecutor(max_workers=N_CORES) as ex:
        xps = list(ex.map(lambda c: pack_x_core(x[c], c64), range(N_CORES)))
    return [{"xp": xps[c], "wt": wt8} for c in range(N_CORES)]


def kernel(x: np.ndarray, weight: np.ndarray) -> np.ndarray:
    from concourse.bass_utils import run_bass_kernel_spmd

    in_maps = make_in_maps(x, weight)
    nc = _get_compiled()
    res = run_bass_kernel_spmd(nc, in_maps, core_ids=list(range(N_CORES)))
    return np.stack(
        [res.results[c]["out"].astype(np.float32) for c in range(N_CORES)],
        axis=0,
    )


# revision 31
# speedup vs baseline: 1.1939x; 1.1939x over previous
"""BitLinear (ternary-weight linear) kernel for Trainium2, 8 NeuronCores.

Computation:  out = x @ (w_ternary * scale)^T
  where scale = max(mean(|weight|), 1e-5)
        w_ternary = clip(round(weight / scale), -1.0, 1.0)  in {-1, 0, 1}

Strategy (data-parallel over batch, 1 batch row per core):
  - Host: quantize the weight to ternary (bit-exact mirror of the jnp
    reference), and split x*(64*scale) into an fp8-e4m3 hi plane plus a
    lo correction plane for k-slices 4..7 only (the partial correction
    halves the quantization-noise variance, measured rel err 1.63e-2
    against the 2e-2 gate, while cutting matmul work 25%).  The 64*scale
    folding keeps the fp8 values ~N(0,1), far from the e4m3 subnormal
    range; the exact power-of-two 2^-6 is unfolded on device in the
    output copy.  Both planes are packed k-major per 128-row block so
    the device needs no transposes or casts at all:
      xp[b, kp, g, s] = plane_g[b*128+s, ks_g*128+kp]
    with g = 0..7 the hi plane (ks = g) and g = 8..11 the lo plane
    (ks = g-4... see LO_SLICES).  Each block slab is a fully contiguous
    192 KB DMA.
  - Device: pure fp8 DoubleRow matmuls (2 k-slices per instruction,
    2 rhs rows/cycle on TRN2 = 157 TF/s): per 128-row block, 6
    stationary lhsT pair-tiles [128, 2, 128], each streamed against both
    512-wide output halves of the un-duplicated fp8 weight (K_eff =
    1536) into two PSUM banks.  The scalar engine copies PSUM -> SBUF
    bf16 with the exact 1/64 scale, and the result DMAs out as bf16
    (upcast to fp32 on host).
"""

import numpy as np
import ml_dtypes

B, S, IN, OUT = 8, 8192, 1024, 1024
N_CORES = 8
P = 128
S_BLOCKS = S // P    # 64
K_TILES = IN // P    # 8
LO_SLICES = [4, 5, 6, 7]   # k-slices that get the fp8 lo correction
G = K_TILES + len(LO_SLICES)  # 12 packed k-groups: hi plane + partial lo
EPS = 1e-5

F8 = ml_dtypes.float8_e4m3
BF16 = ml_dtypes.bfloat16

_compiled = None


def _build():
    import concourse.bacc as bacc
    import concourse.mybir as mybir
    import concourse.tile as tile

    F8D = mybir.dt.float8e4
    F32 = mybir.dt.float32
    BF = mybir.dt.bfloat16
    DR = mybir.MatmulPerfMode.DoubleRow

    nc = bacc.Bacc()
    xp = nc.declare_dram_parameter("xp", [S, G * P], F8D, isOutput=False)
    wt = nc.declare_dram_parameter("wt", [IN, OUT], F8D, isOutput=False)
    out = nc.declare_dram_parameter("out", [S, OUT], BF, isOutput=True)

    # DR pair schedule: (x-group pair start, weight k-slice pair start).
    # Groups 0..7 are the hi plane (ks = g); groups 8..11 are the lo plane
    # for k-slices LO_SLICES = 4..7.
    pairs = [(0, 0), (2, 2), (4, 4), (6, 6), (8, 4), (10, 6)]

    with tile.TileContext(nc) as tc:
        with (
            tc.tile_pool(name="const", bufs=1) as constp,
            tc.tile_pool(name="xn", bufs=3) as xnp,
            tc.tile_pool(name="outp", bufs=8) as outp,
            tc.tile_pool(name="pso", bufs=8, space="PSUM") as pso,
        ):
            def load_x(b):
                t = xnp.tile([P, G, P], F8D, tag="xn", name=f"xn_{b}")
                nc.sync.dma_start(out=t, in_=xp[b * P:(b + 1) * P, :])
                return t

            # Block 0's x goes first on the sync ring; the fp8 weight
            # streams concurrently on the gpsimd ring as 8 fully contiguous
            # 128 KB chunks (each dma_start costs ~650 ns of ring issue
            # time, so fewer, bigger chunks win).
            xt0 = load_x(0)
            wt_sb = constp.tile([P, K_TILES, OUT], F8D)
            for k in range(K_TILES):
                nc.gpsimd.dma_start(
                    out=wt_sb[:, k:k + 1, :],
                    in_=wt[k * P:(k + 1) * P, :],
                )

            for b in range(S_BLOCKS):
                xt = xt0 if b == 0 else load_x(b)

                po = [pso.tile([P, 512], F32, tag="pso", name=f"po{b}_{h}")
                      for h in range(2)]
                # 6 stationary pair-tiles; each serves both output halves
                # back-to-back so weight loads are halved.
                for t, (xg, wk) in enumerate(pairs):
                    for h in range(2):
                        nc.tensor.matmul(
                            po[h],
                            lhsT=xt[:, xg:xg + 2, :],
                            rhs=wt_sb[:, wk:wk + 2, h * 512:(h + 1) * 512],
                            start=(t == 0),
                            stop=(t == len(pairs) - 1),
                            perf_mode=DR,
                        )

                ob = outp.tile([P, OUT], BF, tag="ob", name=f"ob_{b}")
                for h in range(2):
                    nc.scalar.activation(
                        ob[:, h * 512:(h + 1) * 512],
                        po[h],
                        mybir.ActivationFunctionType.Copy,
                        scale=1.0 / 64.0,
                    )
                # outputs alternate between the gpsimd and scalar rings, so
                # the sync ring stays dedicated to the x prefetch and no
                # output backlog builds on one queue.  On the scalar ring
                # the DMA queues right behind this block's Act copies.  The
                # last block rides the by-then-idle sync ring: its software
                # queue finishes in ~0.5 us where gpsimd's hardware queue
                # has a ~4 us end-to-end transfer latency.
                if b == S_BLOCKS - 1:
                    ring = nc.sync
                else:
                    ring = nc.gpsimd if b % 2 == 0 else nc.scalar
                ring.dma_start(out=out[b * P:(b + 1) * P, :], in_=ob)
    nc.finalize()
    return nc


def _get_compiled():
    global _compiled
    if _compiled is None:
        _compiled = _build()
    return _compiled


def quantize_host(weight: np.ndarray):
    """Mirror of the reference ste_quantize, done on host in fp32.

    The mean is computed in float64 then rounded to fp32 so it tracks the
    true mean more closely than any fp32 summation order.
    """
    scale = np.float32(max(np.mean(np.abs(weight), dtype=np.float64), EPS))
    w_t = np.clip(np.round(weight / scale), -1.0, 1.0).astype(np.float32)
    return w_t, scale


def pack_weight(w_t: np.ndarray) -> np.ndarray:
    """Ternary weight [out, in] -> fp8 transposed [in, out]."""
    return np.ascontiguousarray(w_t.T).astype(F8)


def pack_x_core(xc: np.ndarray, c64: np.float32) -> np.ndarray:
    """One core's x [S, IN] fp32 -> packed fp8 hi + partial lo [S, G*P]."""
    xs = xc * c64
    hi = xs.astype(F8)
    lo_cols = np.concatenate(
        [np.arange(sl * P, (sl + 1) * P) for sl in LO_SLICES])
    lo = (xs[:, lo_cols] - hi.astype(np.float32)[:, lo_cols]).astype(F8)
    xp = np.empty((S_BLOCKS, P, G, P), dtype=F8)
    xp[:, :, 0:K_TILES, :] = hi.reshape(
        S_BLOCKS, P, K_TILES, P).transpose(0, 3, 2, 1)
    xp[:, :, K_TILES:G, :] = lo.reshape(
        S_BLOCKS, P, len(LO_SLICES), P).transpose(0, 3, 2, 1)
    return xp.reshape(S, G * P)


def make_in_maps(x: np.ndarray, weight: np.ndarray):
    x = np.asarray(x, dtype=np.float32)
    weight = np.asarray(weight, dtype=np.float32)
    assert x.shape == (B, S, IN) and weight.shape == (OUT, IN)
    w_t, scale = quantize_host(weight)
    wt8 = pack_weight(w_t)
    c64 = np.float32(64.0) * scale
    from concurrent.futures import ThreadPoolExecutor
    with ThreadPoolExecutor(max_workers=N_CORES) as ex:
        xps = list(ex.map(lambda c: pack_x_core(x[c], c64), range(N_CORES)))
    return [{"xp": xps[c], "wt": wt8} for c in range(N_CORES)]


def kernel(x: np.ndarray, weight: np.ndarray) -> np.ndarray:
    from concourse.bass_utils import run_bass_kernel_spmd

    in_maps = make_in_maps(x, weight)
    nc = _get_compiled()
    res = run_bass_kernel_spmd(nc, in_maps, core_ids=list(range(N_CORES)))
    return np.stack(
        [res.results[c]["out"].astype(np.float32) for c in range(N_CORES)],
        axis=0,
    )


# revision 33
# speedup vs baseline: 1.1944x; 1.0004x over previous
"""BitLinear (ternary-weight linear) kernel for Trainium2, 8 NeuronCores.

Computation:  out = x @ (w_ternary * scale)^T
  where scale = max(mean(|weight|), 1e-5)
        w_ternary = clip(round(weight / scale), -1.0, 1.0)  in {-1, 0, 1}

Strategy (data-parallel over batch, 1 batch row per core):
  - Host: quantize the weight to ternary (bit-exact mirror of the jnp
    reference), and split x*(64*scale) into an fp8-e4m3 hi plane plus a
    lo correction plane for k-slices 4..7 only (the partial correction
    halves the quantization-noise variance, measured rel err 1.63e-2
    against the 2e-2 gate, while cutting matmul work 25%).  The 64*scale
    folding keeps the fp8 values ~N(0,1), far from the e4m3 subnormal
    range; the exact power-of-two 2^-6 is unfolded on device in the
    output copy.  Both planes are packed k-major per 128-row block so
    the device needs no transposes or casts at all:
      xp[b, kp, g, s] = plane_g[b*128+s, ks_g*128+kp]
    with g = 0..7 the hi plane (ks = g) and g = 8..11 the lo plane
    (ks = g-4... see LO_SLICES).  Each block slab is a fully contiguous
    192 KB DMA.
  - Device: pure fp8 DoubleRow matmuls (2 k-slices per instruction,
    2 rhs rows/cycle on TRN2 = 157 TF/s): per 128-row block, 6
    stationary lhsT pair-tiles [128, 2, 128], each streamed against both
    512-wide output halves of the un-duplicated fp8 weight (K_eff =
    1536) into two PSUM banks.  The scalar engine copies PSUM -> SBUF
    bf16 with the exact 1/64 scale, and the result DMAs out as bf16
    (upcast to fp32 on host).
"""

import numpy as np
import ml_dtypes

B, S, IN, OUT = 8, 8192, 1024, 1024
N_CORES = 8
P = 128
S_BLOCKS = S // P    # 64
K_TILES = IN // P    # 8
LO_SLICES = [4, 5, 6, 7]   # k-slices that get the fp8 lo correction
G = K_TILES + len(LO_SLICES)  # 12 packed k-groups: hi plane + partial lo
EPS = 1e-5

F8 = ml_dtypes.float8_e4m3
BF16 = ml_dtypes.bfloat16

_compiled = None


def _build():
    import concourse.bacc as bacc
    import concourse.mybir as mybir
    import concourse.tile as tile

    F8D = mybir.dt.float8e4
    F32 = mybir.dt.float32
    BF = mybir.dt.bfloat16
    DR = mybir.MatmulPerfMode.DoubleRow

    nc = bacc.Bacc()
    xp = nc.declare_dram_parameter("xp", [S, G * P], F8D, isOutput=False)
    wt = nc.declare_dram_parameter("wt", [IN, OUT], F8D, isOutput=False)
    out = nc.declare_dram_parameter("out", [S, OUT], BF, isOutput=True)

    # DR pair schedule: (x-group pair start, weight k-slice pair start).
    # Groups 0..7 are the hi plane (ks = g); groups 8..11 are the lo plane
    # for k-slices LO_SLICES = 4..7.
    pairs = [(0, 0), (2, 2), (4, 4), (6, 6), (8, 4), (10, 6)]

    with tile.TileContext(nc) as tc:
        with (
            tc.tile_pool(name="const", bufs=1) as constp,
            tc.tile_pool(name="xn", bufs=3) as xnp,
            tc.tile_pool(name="outp", bufs=8) as outp,
            tc.tile_pool(name="pso", bufs=8, space="PSUM") as pso,
        ):
            def load_x(b):
                t = xnp.tile([P, G, P], F8D, tag="xn", name=f"xn_{b}")
                nc.sync.dma_start(out=t, in_=xp[b * P:(b + 1) * P, :])
                return t

            # Block 0's x goes first on the sync ring; the fp8 weight
            # streams concurrently as 8 fully contiguous 128 KB chunks
            # (each dma_start costs ~650 ns of ring issue time, so fewer,
            # bigger chunks win).  Chunks 0-3 ride the gpsimd ring and 4-7
            # the scalar ring: the DMA engines round-robin across queues,
            # so three concurrent queues get the early-needed chunks in
            # ~2 us sooner than one queue behind a fair 1.2 MB flood.
            xt0 = load_x(0)
            wt_sb = constp.tile([P, K_TILES, OUT], F8D)
            for k in range(K_TILES):
                ring = nc.gpsimd if k < 4 else nc.scalar
                ring.dma_start(
                    out=wt_sb[:, k:k + 1, :],
                    in_=wt[k * P:(k + 1) * P, :],
                )

            for b in range(S_BLOCKS):
                xt = xt0 if b == 0 else load_x(b)

                po = [pso.tile([P, 512], F32, tag="pso", name=f"po{b}_{h}")
                      for h in range(2)]
                # 6 stationary pair-tiles; each serves both output halves
                # back-to-back so weight loads are halved.
                for t, (xg, wk) in enumerate(pairs):
                    for h in range(2):
                        nc.tensor.matmul(
                            po[h],
                            lhsT=xt[:, xg:xg + 2, :],
                            rhs=wt_sb[:, wk:wk + 2, h * 512:(h + 1) * 512],
                            start=(t == 0),
                            stop=(t == len(pairs) - 1),
                            perf_mode=DR,
                        )

                ob = outp.tile([P, OUT], BF, tag="ob", name=f"ob_{b}")
                for h in range(2):
                    nc.scalar.activation(
                        ob[:, h * 512:(h + 1) * 512],
                        po[h],
                        mybir.ActivationFunctionType.Copy,
                        scale=1.0 / 64.0,
                    )
                # outputs alternate between the gpsimd and scalar rings, so
                # the sync ring stays dedicated to the x prefetch and no
                # output backlog builds on one queue.  On the scalar ring
                # the DMA queues right behind this block's Act copies.  The
                # gpsimd hardware queue has a ~4 us end-to-end transfer
                # latency, so the last blocks avoid it entirely and ride
                # the by-then-idle sync ring instead.
                if b >= S_BLOCKS - 6:
                    ring = nc.scalar if b % 2 == 0 else nc.sync
                else:
                    ring = nc.gpsimd if b % 2 == 0 else nc.scalar
                ring.dma_start(out=out[b * P:(b + 1) * P, :], in_=ob)
    nc.finalize()
    return nc


def _get_compiled():
    global _compiled
    if _compiled is None:
        _compiled = _build()
    return _compiled


def quantize_host(weight: np.ndarray):
    """Mirror of the reference ste_quantize, done on host in fp32.

    The mean is computed in float64 then rounded to fp32 so it tracks the
    true mean more closely than any fp32 summation order.
    """
    scale = np.float32(max(np.mean(np.abs(weight), dtype=np.float64), EPS))
    w_t = np.clip(np.round(weight / scale), -1.0, 1.0).astype(np.float32)
    return w_t, scale


def pack_weight(w_t: np.ndarray) -> np.ndarray:
    """Ternary weight [out, in] -> fp8 transposed [in, out]."""
    return np.ascontiguousarray(w_t.T).astype(F8)


def pack_x_core(xc: np.ndarray, c64: np.float32) -> np.ndarray:
    """One core's x [S, IN] fp32 -> packed fp8 hi + partial lo [S, G*P]."""
    xs = xc * c64
    hi = xs.astype(F8)
    lo_cols = np.concatenate(
        [np.arange(sl * P, (sl + 1) * P) for sl in LO_SLICES])
    lo = (xs[:, lo_cols] - hi.astype(np.float32)[:, lo_cols]).astype(F8)
    xp = np.empty((S_BLOCKS, P, G, P), dtype=F8)
    xp[:, :, 0:K_TILES, :] = hi.reshape(
        S_BLOCKS, P, K_TILES, P).transpose(0, 3, 2, 1)
    xp[:, :, K_TILES:G, :] = lo.reshape(
        S_BLOCKS, P, len(LO_SLICES), P).transpose(0, 3, 2, 1)
    return xp.reshape(S, G * P)


def make_in_maps(x: np.ndarray, weight: np.ndarray):
    x = np.asarray(x, dtype=np.float32)
    weight = np.asarray(weight, dtype=np.float32)
    assert x.shape == (B, S, IN) and weight.shape == (OUT, IN)
    w_t, scale = quantize_host(weight)
    wt8 = pack_weight(w_t)
    c64 = np.float32(64.0) * scale
    from concurrent.futures import ThreadPoolExecutor
    with ThreadPoolExecutor(max_workers=N_CORES) as ex:
        xps = list(ex.map(lambda c: pack_x_core(x[c], c64), range(N_CORES)))
    return [{"xp": xps[c], "wt": wt8} for c in range(N_CORES)]


def kernel(x: np.ndarray, weight: np.ndarray) -> np.ndarray:
    from concourse.bass_utils import run_bass_kernel_spmd

    in_maps = make_in_maps(x, weight)
    nc = _get_compiled()
    res = run_bass_kernel_spmd(nc, in_maps, core_ids=list(range(N_CORES)))
    return np.stack(
        [res.results[c]["out"].astype(np.float32) for c in range(N_CORES)],
        axis=0,
    )
